# revision 43
# baseline (speedup 1.0000x reference)
"""Trainium2 Bass kernel for nn_ClassificationLoss (NMS-detection CE loss).

Data-parallel across 8 NeuronCores; each core handles 2 of the 16 images,
STACKED on the partition dim: partitions 0-62 hold image A's 25200 preds
(400 per partition), partitions 63-125 hold image B's.

Math reformulation (validated against the reference in fp16):
  valid:  max_m iou >= 0.4  <=>  max_m [inter_m - (2/7)*ga_m] >= (2/7)*pa
          (exact algebra; no division anywhere)
  label:  gcls[argmax_m (inter_m - (2/7)*ga_m)]  (approximates argmax iou;
          measured rel-err vs reference 1.3e-4, far under the 2e-2 gate)
  ce    = ln(sum_c exp(s_c)) - ln(exp(s_label))  (logits ~N(0,1))

Engine split per 25-pred chunk (fp16 pairwise tiles [126, 64, 25], packed
last dim so tensor_tensor runs 2x and tensor_scalar 4x on the DVE):
  DVE:   min/max/sub chain, relu*relu, t, tree-max, eq/lw/tree-sum label,
         tail of the exp-sum tree
  Act:   exp(scores) -> fp16, final Ln's
  Pool:  first exp-sum tree level, per-pred score pick via indirect_copy
  PE:    idle; DMA: chunk loads (the memory roofline)
"""

import numpy as np

import concourse.bass as bass
import concourse.bacc as bacc
import concourse.tile as tile
import concourse.mybir as mybir
from concourse.bass_utils import run_bass_kernel_spmd

B, N, C, M = 16, 25200, 80, 64
NCORES = 8
IMGS = B // NCORES                   # 2 images per core
P = 126                              # partitions; 63 per image
HP = P // IMGS                       # 63
RPP = IMGS * N // P                  # 400 preds per partition
KC = 25                              # preds per chunk (per partition)
NCH = RPP // KC                      # 16 chunks
CTH = float(np.float32(2.0) / np.float32(7.0))

F32 = mybir.dt.float32
F16 = mybir.dt.float16
U16 = mybir.dt.uint16
Alu = mybir.AluOpType
Act = mybir.ActivationFunctionType
AX = mybir.AxisListType

_CACHE = {}


def _bc(ap_like, extra_offset, dims):
    """Raw AP with explicit [step, count] dims (0-step = broadcast)."""
    return bass.AP(tensor=ap_like.tensor, offset=ap_like.offset + extra_offset, ap=dims)


def _kb(t, w=KC):
    """[P, (0,M), (1,w)] view of a [P, w] tile: per-pred value bcast over M."""
    a = t[:, :]
    return _bc(a, 0, [a.ap[0], [0, M], [1, w]])


def _build():
    nc = bacc.Bacc("TRN2")
    p_in = nc.dram_tensor("p", [P, RPP, 85], F32, kind="ExternalInput")
    g_in = nc.dram_tensor("g", [IMGS, M, 5], F32, kind="ExternalInput")
    o_out = nc.dram_tensor("o", [P, 2], F32, kind="ExternalOutput")

    with tile.TileContext(nc) as tc:
        with (
            tc.tile_pool(name="chunkp", bufs=2) as chunkp,
            tc.tile_pool(name="singles", bufs=1) as singles,
            tc.tile_pool(name="scr", bufs=2) as scr,
            tc.tile_pool(name="escp", bufs=3) as escp,
            tc.tile_pool(name="lwp", bufs=4) as lwp,
            tc.tile_pool(name="bufp", bufs=1) as bufp,
        ):
            # ---------------- GT prep (once; both images via partition halves)
            graw = singles.tile([P, M, 5], F32)
            nc.gpsimd.dma_start(
                out=graw[0:HP], in_=_bc(g_in[:], 0, [[0, HP], [5, M], [1, 5]])
            )
            nc.gpsimd.dma_start(
                out=graw[HP:P], in_=_bc(g_in[:], M * 5, [[0, HP], [5, M], [1, 5]])
            )
            # fp16 per-coord rows [P, M]
            gh = {}
            for name, col in (("x1", 0), ("y1", 1), ("x2", 2), ("y2", 3), ("cl", 4)):
                t = singles.tile([P, M], F16, tag=f"gh{name}")
                nc.vector.tensor_copy(t, graw[:, :, col])
                gh[name] = t
            d1 = singles.tile([P, M], F32, tag="d1")
            d2 = singles.tile([P, M], F32, tag="d2")
            ga = singles.tile([P, M], F32, tag="ga")
            nc.vector.tensor_tensor(d1, graw[:, :, 2], graw[:, :, 0], op=Alu.subtract)
            nc.vector.tensor_tensor(d2, graw[:, :, 3], graw[:, :, 1], op=Alu.subtract)
            nc.vector.tensor_tensor(ga, d1, d2, op=Alu.mult)
            gac = singles.tile([P, M], F16, tag="gac")
            nc.vector.tensor_scalar(gac, ga, CTH, None, op0=Alu.mult)

            # materialized [P, M, KC] fp16 K-broadcast tiles (reused all chunks)
            def kmat(src, tagn, eng=None):
                t = singles.tile([P, M, KC], F16, tag=tagn)
                a = src[:, :]
                ap = _bc(a, 0, [a.ap[0], [1, M], [0, KC]])
                if eng == "act":
                    nc.scalar.activation(t, ap, Act.Copy)
                else:
                    nc.vector.tensor_copy(t, ap)
                return t

            gx1K = kmat(gh["x1"], "gx1K")
            gy1K = kmat(gh["y1"], "gy1K")
            gx2K = kmat(gh["x2"], "gx2K")
            gy2K = kmat(gh["y2"], "gy2K")
            gacK = None
            gclK = None

            # uint16 iota k*C for flat-index into [KC, C] score tiles
            iotaK = singles.tile([P, KC], U16)
            nc.gpsimd.iota(iotaK, pattern=[[C, KC]], base=0, channel_multiplier=0)

            # ---------------- per-pred accumulators [P, RPP]
            seb = bufp.tile([P, RPP], F32, tag="seb")     # sum exp
            valb = bufp.tile([P, RPP], F16, tag="valb")   # valid mask
            # gather path needs partition dim % 16 == 0 -> 128-row tiles,
            # rows 126..127 are junk (memset once, never read by the host)
            eslb = bufp.tile([128, RPP], F16, tag="eslb")  # exp(s_label)
            esc_b0 = singles.tile([128, KC, C], F16, tag="escb0")
            esc_b1 = singles.tile([128, KC, C], F16, tag="escb1")
            esc_b2 = singles.tile([128, KC, C], F16, tag="escb2")
            idx_b0 = singles.tile([128, KC], U16, tag="idxb0")
            idx_b1 = singles.tile([128, KC], U16, tag="idxb1")
            idx_b2 = singles.tile([128, KC], U16, tag="idxb2")
            esc_bufs = [esc_b0, esc_b1, esc_b2]
            idx_bufs = [idx_b0, idx_b1, idx_b2]
            for i in range(3):
                nc.gpsimd.memset(esc_bufs[i], 1.0)
                nc.gpsimd.memset(idx_bufs[i], 0)
            nc.gpsimd.memset(eslb, 1.0)

            # software-pipelined emission: A1 (load/chain), A2 (iou/eq),
            # B (label-sum/gather/exp-tail), skewed so no engine stalls on a
            # same-chunk cross-engine dependency.
            lw_t = {}
            ck_t = {}
            tmax_t = {}
            t_t = {}
            e40_t = {}

            def emit_A1(j):
                c0 = j * KC
                ck = chunkp.tile([P, KC, 85], F32, tag="ck")
                nc.sync.dma_start(out=ck, in_=p_in[:, c0:c0 + KC, :])
                ck_t[j] = ck
                pxy = {}
                for name, col in (("x1", 0), ("y1", 1), ("x2", 2), ("y2", 3)):
                    t = scr.tile([P, KC], F16, tag=f"p{name}")
                    nc.scalar.activation(t, ck[:, :, col], Act.Copy)
                    pxy[name] = t
                wd = scr.tile([P, KC, 2], F32, tag="wd")
                nc.gpsimd.tensor_tensor(wd, ck[:, :, 2:4], ck[:, :, 0:2], op=Alu.subtract)
                pa32 = scr.tile([P, KC], F32, tag="pa32")
                nc.gpsimd.tensor_tensor(pa32, wd[:, :, 0], wd[:, :, 1], op=Alu.mult)
                pac = scr.tile([P, KC], F16, tag="pac")
                nc.scalar.activation(pac, pa32, Act.Copy, scale=CTH)
                u = scr.tile([P, M, KC], F16, tag="t_u")
                v = scr.tile([P, M, KC], F16, tag="t_v")
                wx = scr.tile([P, M, KC], F16, tag="t_wx")
                nc.vector.tensor_tensor(u, gx2K, _kb(pxy["x2"]), op=Alu.min)
                nc.vector.tensor_tensor(v, gx1K, _kb(pxy["x1"]), op=Alu.max)
                nc.vector.tensor_tensor(wx, u, v, op=Alu.subtract)
                uy = scr.tile([P, M, KC], F16, tag="t_uy")
                vy = scr.tile([P, M, KC], F16, tag="t_vy")
                wy = scr.tile([P, M, KC], F16, tag="t_wy")
                nc.vector.tensor_tensor(uy, gy2K, _kb(pxy["y2"]), op=Alu.min)
                nc.vector.tensor_tensor(vy, gy1K, _kb(pxy["y1"]), op=Alu.max)
                nc.vector.tensor_tensor(wy, uy, vy, op=Alu.subtract)
                rwx = scr.tile([P, M, KC], F16, tag="t_rwx")
                rwy = scr.tile([P, M, KC], F16, tag="t_rwy")
                nc.scalar.activation(rwx, wx, Act.Relu)
                nc.scalar.activation(rwy, wy, Act.Relu)
                # exp side-chain starts here too (ACT)
                esc = esc_bufs[j % 3]
                nc.scalar.activation(esc[0:P], ck[:, :, 5:85], Act.Exp)
                return pxy, pac, rwx, rwy

            def emit_A2(j, st):
                c0 = j * KC
                assert gacK is not None
                pxy, pac, rwx, rwy = st
                inter = scr.tile([P, M, KC], F16, tag="t_in")
                nc.vector.tensor_tensor(inter, rwx, rwy, op=Alu.mult)
                t_ = scr.tile([P, M, KC], F16, tag="t_t")
                nc.vector.tensor_tensor(t_, inter, gacK, op=Alu.subtract)
                t_t[j] = t_
                tm = t_
                wdt = M
                while wdt > 1:
                    h = wdt // 2
                    nt = treep.tile([P, h, KC], F16, tag=f"tm{h}")
                    nc.vector.tensor_tensor(nt, tm[:, 0:h, :], tm[:, h:wdt, :], op=Alu.max)
                    tm = nt
                    wdt = h
                tmax_t[j] = tm
                nc.vector.tensor_tensor(
                    valb[:, c0:c0 + KC], tm[:, 0, :], pac, op=Alu.is_ge
                )
                tmaxb = _bc(tm[:, 0, :], 0, [tm[:, 0, :].ap[0], [0, M], [1, KC]])
                eq = scr.tile([P, M, KC], F16, tag="t_eq")
                nc.vector.tensor_tensor(eq, t_, tmaxb, op=Alu.is_equal)
                lw = lwp.tile([P, M, KC], F16, tag="t_lw")
                lw_eng = nc.vector if j >= NCH - 2 else nc.gpsimd
                lw_eng.tensor_tensor(lw, eq, gclK, op=Alu.mult)
                lw_t[j] = lw
                esc = esc_bufs[j % 3]
                e40 = escp.tile([P, KC, 40], F16, tag="e40")
                e_eng = nc.vector if j >= NCH - 2 else nc.gpsimd
                e_eng.tensor_tensor(
                    e40, esc[0:P, :, 0:40], esc[0:P, :, 40:80], op=Alu.add
                )
                e40_t[j] = e40

            def emit_B(j):
                c0 = j * KC
                sm = lw_t.pop(j)
                wdt = M
                first = True
                while wdt > 1:
                    h = wdt // 2
                    nt = treep.tile([P, h, KC], F16, tag=f"ts{h}")
                    nc.vector.tensor_tensor(nt, sm[:, 0:h, :], sm[:, h:wdt, :], op=Alu.add)
                    sm = nt
                    wdt = h
                    first = False
                labu = scr.tile([P, KC], U16, tag="labu")
                if j >= NCH - 2:
                    nc.vector.tensor_copy(labu, sm[:, 0, :])
                else:
                    nc.scalar.activation(labu, sm[:, 0, :], Act.Copy)
                idx = idx_bufs[j % 3]
                nc.vector.tensor_tensor(idx[0:P], labu, iotaK, op=Alu.add)
                esc = esc_bufs[j % 3]
                esc2d = esc[:, :, :]
                esc2d = bass.AP(
                    tensor=esc2d.tensor, offset=esc2d.offset,
                    ap=[esc2d.ap[0], [1, KC * C]],
                )
                nc.gpsimd.indirect_copy(
                    eslb[:, c0:c0 + KC], esc2d, idx[:, :],
                    i_know_ap_gather_is_preferred=True,
                )
                e40 = e40_t.pop(j)
                e20 = treep.tile([P, KC, 20], F16, tag="e20")
                e_eng = nc.vector if j >= NCH - 2 else nc.gpsimd
                e_eng.tensor_tensor(e20, e40[:, :, 0:20], e40[:, :, 20:40], op=Alu.add)
                e10 = treep.tile([P, KC, 10], F16, tag="e10")
                nc.vector.tensor_tensor(e10, e20[:, :, 0:10], e20[:, :, 10:20], op=Alu.add)
                e5 = treep.tile([P, KC, 5], F16, tag="e5")
                nc.vector.tensor_tensor(e5, e10[:, :, 0:5], e10[:, :, 5:10], op=Alu.add)
                e21 = treep.tile([P, KC, 2], F16, tag="e21")
                nc.vector.tensor_tensor(e21, e5[:, :, 0:2], e5[:, :, 2:4], op=Alu.add)
                e11 = treep.tile([P, KC, 1], F16, tag="e11")
                nc.vector.tensor_tensor(e11, e21[:, :, 0:1], e21[:, :, 1:2], op=Alu.add)
                nc.vector.tensor_tensor(
                    seb[:, c0:c0 + KC], e11[:, :, 0], e5[:, :, 4], op=Alu.add
                )

            stash = {}
            stash[0] = emit_A1(0)
            stash[1] = emit_A1(1)
            # needed first in A2(0)/B(0) -- emitted late so ACT serves chunk-0
            # coord copies before these one-time broadcasts
            gacK = kmat(gac, "gacK", "act")
            gclK = kmat(gh["cl"], "gclK", "act")
            emit_A2(0, stash.pop(0))
            stash[2] = emit_A1(2)
            emit_A2(1, stash.pop(1))
            for j in range(3, NCH):
                stash[j] = emit_A1(j)
                emit_A2(j - 1, stash.pop(j - 1))
                emit_B(j - 3)
            emit_A2(NCH - 1, stash.pop(NCH - 1))
            emit_B(NCH - 3)
            emit_B(NCH - 2)
            emit_B(NCH - 1)

            # ---------------- epilogue
            lseb = bufp.tile([P, RPP], F16, tag="lseb")
            leslb = bufp.tile([P, RPP], F16, tag="leslb")
            nc.scalar.activation(lseb, seb, Act.Ln)
            nc.scalar.activation(leslb, eslb[0:P], Act.Ln)
            ce = bufp.tile([P, RPP], F16, tag="ce")
            nc.gpsimd.tensor_tensor(ce, lseb, leslb, op=Alu.subtract)
            cev = bufp.tile([P, RPP], F16, tag="cev")
            nc.gpsimd.tensor_tensor(cev, ce, valb, op=Alu.mult)
            out_t = singles.tile([P, 2], F32)
            nc.vector.reduce_sum(out_t[:, 0:1], cev, axis=AX.X)
            nc.vector.reduce_sum(out_t[:, 1:2], valb, axis=AX.X)
            nc.sync.dma_start(out=o_out[:], in_=out_t)

    nc.compile()
    return nc


def kernel(preds: np.ndarray, gtruths: np.ndarray) -> np.ndarray:
    if "nc" not in _CACHE:
        _CACHE["nc"] = _build()
    nc = _CACHE["nc"]

    preds = np.ascontiguousarray(preds, dtype=np.float32)
    gtruths = np.ascontiguousarray(gtruths, dtype=np.float32)
    in_maps = [
        {
            "p": preds[c * IMGS:(c + 1) * IMGS].reshape(P, RPP, 85),
            "g": gtruths[c * IMGS:(c + 1) * IMGS],
        }
        for c in range(NCORES)
    ]
    res = run_bass_kernel_spmd(nc, in_maps, core_ids=list(range(NCORES)))
    _CACHE["last_result"] = res

    per_img = []
    for c in range(NCORES):
        o = res.results[c]["o"]  # [P, 2]
        for b in range(IMGS):
            half = o[b * HP:(b + 1) * HP]
            ce_sum = float(half[:, 0].sum(dtype=np.float64))
            cnt = float(half[:, 1].sum(dtype=np.float64))
            per_img.append(ce_sum / max(cnt, 1.0))
    return np.asarray(np.mean(per_img), dtype=np.float32)


# revision 44
# speedup vs baseline: 1.0058x; 1.0058x over previous
"""Trainium2 Bass kernel for nn_ClassificationLoss (NMS-detection CE loss).

Data-parallel across 8 NeuronCores; each core handles 2 of the 16 images,
STACKED on the partition dim: partitions 0-62 hold image A's 25200 preds
(400 per partition), partitions 63-125 hold image B's.

Math reformulation (validated against the reference in fp16):
  valid:  max_m iou >= 0.4  <=>  max_m [inter_m - (2/7)*ga_m] >= (2/7)*pa
          (exact algebra; no division anywhere)
  label:  gcls[argmax_m (inter_m - (2/7)*ga_m)]  (approximates argmax iou;
          measured rel-err vs reference 1.3e-4, far under the 2e-2 gate)
  ce    = ln(sum_c exp(s_c)) - ln(exp(s_label))  (logits ~N(0,1))

Engine split per 25-pred chunk (fp16 pairwise tiles [126, 64, 25], packed
last dim so tensor_tensor runs 2x and tensor_scalar 4x on the DVE):
  DVE:   min/max/sub chain, relu*relu, t, tree-max, eq/lw/tree-sum label,
         tail of the exp-sum tree
  Act:   exp(scores) -> fp16, final Ln's
  Pool:  first exp-sum tree level, per-pred score pick via indirect_copy
  PE:    idle; DMA: chunk loads (the memory roofline)
"""

import numpy as np

import concourse.bass as bass
import concourse.bacc as bacc
import concourse.tile as tile
import concourse.mybir as mybir
from concourse.bass_utils import run_bass_kernel_spmd

B, N, C, M = 16, 25200, 80, 64
NCORES = 8
IMGS = B // NCORES                   # 2 images per core
P = 126                              # partitions; 63 per image
HP = P // IMGS                       # 63
RPP = IMGS * N // P                  # 400 preds per partition
KC = 25                              # preds per chunk (per partition)
NCH = RPP // KC                      # 16 chunks
CTH = float(np.float32(2.0) / np.float32(7.0))

F32 = mybir.dt.float32
F16 = mybir.dt.float16
U16 = mybir.dt.uint16
Alu = mybir.AluOpType
Act = mybir.ActivationFunctionType
AX = mybir.AxisListType

_CACHE = {}


def _bc(ap_like, extra_offset, dims):
    """Raw AP with explicit [step, count] dims (0-step = broadcast)."""
    return bass.AP(tensor=ap_like.tensor, offset=ap_like.offset + extra_offset, ap=dims)


def _kb(t, w=KC):
    """[P, (0,M), (1,w)] view of a [P, w] tile: per-pred value bcast over M."""
    a = t[:, :]
    return _bc(a, 0, [a.ap[0], [0, M], [1, w]])


def _build():
    nc = bacc.Bacc("TRN2")
    p_in = nc.dram_tensor("p", [P, RPP, 85], F32, kind="ExternalInput")
    g_in = nc.dram_tensor("g", [IMGS, M, 5], F32, kind="ExternalInput")
    o_out = nc.dram_tensor("o", [P, 2], F32, kind="ExternalOutput")

    with tile.TileContext(nc) as tc:
        with (
            tc.tile_pool(name="chunkp", bufs=2) as chunkp,
            tc.tile_pool(name="singles", bufs=1) as singles,
            tc.tile_pool(name="scr", bufs=2) as scr,
            tc.tile_pool(name="escp", bufs=3) as escp,
            tc.tile_pool(name="lwp", bufs=4) as lwp,
            tc.tile_pool(name="bufp", bufs=1) as bufp,
        ):
            # ---------------- GT prep (once; both images via partition halves)
            graw = singles.tile([P, M, 5], F32)
            nc.gpsimd.dma_start(
                out=graw[0:HP], in_=_bc(g_in[:], 0, [[0, HP], [5, M], [1, 5]])
            )
            nc.gpsimd.dma_start(
                out=graw[HP:P], in_=_bc(g_in[:], M * 5, [[0, HP], [5, M], [1, 5]])
            )
            # fp16 per-coord rows [P, M]
            gh = {}
            for name, col in (("x1", 0), ("y1", 1), ("x2", 2), ("y2", 3), ("cl", 4)):
                t = singles.tile([P, M], F16, tag=f"gh{name}")
                nc.vector.tensor_copy(t, graw[:, :, col])
                gh[name] = t
            d1 = singles.tile([P, M], F32, tag="d1")
            d2 = singles.tile([P, M], F32, tag="d2")
            ga = singles.tile([P, M], F32, tag="ga")
            nc.vector.tensor_tensor(d1, graw[:, :, 2], graw[:, :, 0], op=Alu.subtract)
            nc.vector.tensor_tensor(d2, graw[:, :, 3], graw[:, :, 1], op=Alu.subtract)
            nc.vector.tensor_tensor(ga, d1, d2, op=Alu.mult)
            gac = singles.tile([P, M], F16, tag="gac")
            nc.vector.tensor_scalar(gac, ga, CTH, None, op0=Alu.mult)

            # materialized [P, M, KC] fp16 K-broadcast tiles (reused all chunks)
            def kmat(src, tagn, eng=None):
                t = singles.tile([P, M, KC], F16, tag=tagn)
                a = src[:, :]
                ap = _bc(a, 0, [a.ap[0], [1, M], [0, KC]])
                if eng == "act":
                    nc.scalar.activation(t, ap, Act.Copy)
                else:
                    nc.vector.tensor_copy(t, ap)
                return t

            gx1K = kmat(gh["x1"], "gx1K")
            gy1K = kmat(gh["y1"], "gy1K")
            gx2K = kmat(gh["x2"], "gx2K")
            gy2K = kmat(gh["y2"], "gy2K")
            gacK = None
            gclK = None

            # uint16 iota k*C for flat-index into [KC, C] score tiles
            iotaK = singles.tile([P, KC], U16)
            nc.gpsimd.iota(iotaK, pattern=[[C, KC]], base=0, channel_multiplier=0)

            # ---------------- per-pred accumulators [P, RPP]
            seb = bufp.tile([P, RPP], F32, tag="seb")     # sum exp
            valb = bufp.tile([P, RPP], F16, tag="valb")   # valid mask
            # gather path needs partition dim % 16 == 0 -> 128-row tiles,
            # rows 126..127 are junk (memset once, never read by the host)
            eslb = bufp.tile([128, RPP], F16, tag="eslb")  # exp(s_label)
            esc_b0 = singles.tile([128, KC, C], F16, tag="escb0")
            esc_b1 = singles.tile([128, KC, C], F16, tag="escb1")
            esc_b2 = singles.tile([128, KC, C], F16, tag="escb2")
            idx_b0 = singles.tile([128, KC], U16, tag="idxb0")
            idx_b1 = singles.tile([128, KC], U16, tag="idxb1")
            idx_b2 = singles.tile([128, KC], U16, tag="idxb2")
            esc_bufs = [esc_b0, esc_b1, esc_b2]
            idx_bufs = [idx_b0, idx_b1, idx_b2]
            for i in range(3):
                nc.gpsimd.memset(esc_bufs[i], 1.0)
                nc.gpsimd.memset(idx_bufs[i], 0)
            nc.gpsimd.memset(eslb, 1.0)

            # software-pipelined emission: A1 (load/chain), A2 (iou/eq),
            # B (label-sum/gather/exp-tail), skewed so no engine stalls on a
            # same-chunk cross-engine dependency.
            lw_t = {}
            ck_t = {}
            tmax_t = {}
            t_t = {}
            e40_t = {}

            def emit_A1(j):
                c0 = j * KC
                ck = chunkp.tile([P, KC, 85], F32, tag="ck")
                nc.sync.dma_start(out=ck, in_=p_in[:, c0:c0 + KC, :])
                ck_t[j] = ck
                pxy = {}
                for name, col in (("x1", 0), ("y1", 1), ("x2", 2), ("y2", 3)):
                    t = scr.tile([P, KC], F16, tag=f"p{name}")
                    nc.scalar.activation(t, ck[:, :, col], Act.Copy)
                    pxy[name] = t
                wd = scr.tile([P, KC, 2], F32, tag="wd")
                nc.gpsimd.tensor_tensor(wd, ck[:, :, 2:4], ck[:, :, 0:2], op=Alu.subtract)
                pa32 = scr.tile([P, KC], F32, tag="pa32")
                nc.gpsimd.tensor_tensor(pa32, wd[:, :, 0], wd[:, :, 1], op=Alu.mult)
                pac = scr.tile([P, KC], F16, tag="pac")
                nc.scalar.activation(pac, pa32, Act.Copy, scale=CTH)
                u = scr.tile([P, M, KC], F16, tag="t_u")
                v = scr.tile([P, M, KC], F16, tag="t_v")
                wx = scr.tile([P, M, KC], F16, tag="t_wx")
                nc.vector.tensor_tensor(u, gx2K, _kb(pxy["x2"]), op=Alu.min)
                nc.vector.tensor_tensor(v, gx1K, _kb(pxy["x1"]), op=Alu.max)
                nc.vector.tensor_tensor(wx, u, v, op=Alu.subtract)
                uy = scr.tile([P, M, KC], F16, tag="t_uy")
                vy = scr.tile([P, M, KC], F16, tag="t_vy")
                wy = scr.tile([P, M, KC], F16, tag="t_wy")
                nc.vector.tensor_tensor(uy, gy2K, _kb(pxy["y2"]), op=Alu.min)
                nc.vector.tensor_tensor(vy, gy1K, _kb(pxy["y1"]), op=Alu.max)
                nc.vector.tensor_tensor(wy, uy, vy, op=Alu.subtract)
                rwx = scr.tile([P, M, KC], F16, tag="t_rwx")
                rwy = scr.tile([P, M, KC], F16, tag="t_rwy")
                nc.scalar.activation(rwx, wx, Act.Relu)
                nc.scalar.activation(rwy, wy, Act.Relu)
                # exp side-chain starts here too (ACT)
                esc = esc_bufs[j % 3]
                nc.scalar.activation(esc[0:P], ck[:, :, 5:85], Act.Exp)
                return pxy, pac, rwx, rwy

            def emit_A2(j, st):
                c0 = j * KC
                assert gacK is not None
                pxy, pac, rwx, rwy = st
                inter = scr.tile([P, M, KC], F16, tag="t_in")
                nc.vector.tensor_tensor(inter, rwx, rwy, op=Alu.mult)
                t_ = scr.tile([P, M, KC], F16, tag="t_t")
                nc.vector.tensor_tensor(t_, inter, gacK, op=Alu.subtract)
                t_t[j] = t_
                tm = t_
                wdt = M
                while wdt > 1:
                    h = wdt // 2
                    nt = treep.tile([P, h, KC], F16, tag=f"tm{h}")
                    nc.vector.tensor_tensor(nt, tm[:, 0:h, :], tm[:, h:wdt, :], op=Alu.max)
                    tm = nt
                    wdt = h
                tmax_t[j] = tm
                nc.vector.tensor_tensor(
                    valb[:, c0:c0 + KC], tm[:, 0, :], pac, op=Alu.is_ge
                )
                tmaxb = _bc(tm[:, 0, :], 0, [tm[:, 0, :].ap[0], [0, M], [1, KC]])
                eq = scr.tile([P, M, KC], F16, tag="t_eq")
                nc.vector.tensor_tensor(eq, t_, tmaxb, op=Alu.is_equal)
                lw = lwp.tile([P, M, KC], F16, tag="t_lw")
                lw_eng = nc.vector if j >= NCH - 2 else nc.gpsimd
                lw_eng.tensor_tensor(lw, eq, gclK, op=Alu.mult)
                lw_t[j] = lw
                esc = esc_bufs[j % 3]
                e40 = escp.tile([P, KC, 40], F16, tag="e40")
                nc.gpsimd.tensor_tensor(
                    e40, esc[0:P, :, 0:40], esc[0:P, :, 40:80], op=Alu.add
                )
                e40_t[j] = e40

            def emit_B(j):
                c0 = j * KC
                sm = lw_t.pop(j)
                wdt = M
                first = True
                while wdt > 1:
                    h = wdt // 2
                    nt = treep.tile([P, h, KC], F16, tag=f"ts{h}")
                    nc.vector.tensor_tensor(nt, sm[:, 0:h, :], sm[:, h:wdt, :], op=Alu.add)
                    sm = nt
                    wdt = h
                    first = False
                labu = scr.tile([P, KC], U16, tag="labu")
                if j >= NCH - 2:
                    nc.vector.tensor_copy(labu, sm[:, 0, :])
                else:
                    nc.scalar.activation(labu, sm[:, 0, :], Act.Copy)
                idx = idx_bufs[j % 3]
                nc.vector.tensor_tensor(idx[0:P], labu, iotaK, op=Alu.add)
                esc = esc_bufs[j % 3]
                esc2d = esc[:, :, :]
                esc2d = bass.AP(
                    tensor=esc2d.tensor, offset=esc2d.offset,
                    ap=[esc2d.ap[0], [1, KC * C]],
                )
                nc.gpsimd.indirect_copy(
                    eslb[:, c0:c0 + KC], esc2d, idx[:, :],
                    i_know_ap_gather_is_preferred=True,
                )
                e40 = e40_t.pop(j)
                e20 = treep.tile([P, KC, 20], F16, tag="e20")
                nc.gpsimd.tensor_tensor(e20, e40[:, :, 0:20], e40[:, :, 20:40], op=Alu.add)
                e10 = treep.tile([P, KC, 10], F16, tag="e10")
                nc.vector.tensor_tensor(e10, e20[:, :, 0:10], e20[:, :, 10:20], op=Alu.add)
                e5 = treep.tile([P, KC, 5], F16, tag="e5")
                nc.vector.tensor_tensor(e5, e10[:, :, 0:5], e10[:, :, 5:10], op=Alu.add)
                e21 = treep.tile([P, KC, 2], F16, tag="e21")
                nc.vector.tensor_tensor(e21, e5[:, :, 0:2], e5[:, :, 2:4], op=Alu.add)
                e11 = treep.tile([P, KC, 1], F16, tag="e11")
                nc.vector.tensor_tensor(e11, e21[:, :, 0:1], e21[:, :, 1:2], op=Alu.add)
                nc.vector.tensor_tensor(
                    seb[:, c0:c0 + KC], e11[:, :, 0], e5[:, :, 4], op=Alu.add
                )

            stash = {}
            stash[0] = emit_A1(0)
            stash[1] = emit_A1(1)
            # needed first in A2(0)/B(0) -- emitted late so ACT serves chunk-0
            # coord copies before these one-time broadcasts
            gacK = kmat(gac, "gacK", "act")
            gclK = kmat(gh["cl"], "gclK", "act")
            emit_A2(0, stash.pop(0))
            stash[2] = emit_A1(2)
            emit_A2(1, stash.pop(1))
            for j in range(3, NCH):
                stash[j] = emit_A1(j)
                emit_A2(j - 1, stash.pop(j - 1))
                emit_B(j - 3)
            emit_A2(NCH - 1, stash.pop(NCH - 1))
            emit_B(NCH - 3)
            emit_B(NCH - 2)
            emit_B(NCH - 1)

            # ---------------- epilogue
            lseb = bufp.tile([P, RPP], F16, tag="lseb")
            leslb = bufp.tile([P, RPP], F16, tag="leslb")
            nc.scalar.activation(lseb, seb, Act.Ln)
            nc.scalar.activation(leslb, eslb[0:P], Act.Ln)
            ce = bufp.tile([P, RPP], F16, tag="ce")
            nc.gpsimd.tensor_tensor(ce, lseb, leslb, op=Alu.subtract)
            cev = bufp.tile([P, RPP], F16, tag="cev")
            nc.gpsimd.tensor_tensor(cev, ce, valb, op=Alu.mult)
            out_t = singles.tile([P, 2], F32)
            nc.vector.reduce_sum(out_t[:, 0:1], cev, axis=AX.X)
            nc.vector.reduce_sum(out_t[:, 1:2], valb, axis=AX.X)
            nc.sync.dma_start(out=o_out[:], in_=out_t)

    nc.compile()
    return nc


def kernel(preds: np.ndarray, gtruths: np.ndarray) -> np.ndarray:
    if "nc" not in _CACHE:
        _CACHE["nc"] = _build()
    nc = _CACHE["nc"]

    preds = np.ascontiguousarray(preds, dtype=np.float32)
    gtruths = np.ascontiguousarray(gtruths, dtype=np.float32)
    in_maps = [
        {
            "p": preds[c * IMGS:(c + 1) * IMGS].reshape(P, RPP, 85),
            "g": gtruths[c * IMGS:(c + 1) * IMGS],
        }
        for c in range(NCORES)
    ]
    res = run_bass_kernel_spmd(nc, in_maps, core_ids=list(range(NCORES)))
    _CACHE["last_result"] = res

    per_img = []
    for c in range(NCORES):
        o = res.results[c]["o"]  # [P, 2]
        for b in range(IMGS):
            half = o[b * HP:(b + 1) * HP]
            ce_sum = float(half[:, 0].sum(dtype=np.float64))
            cnt = float(half[:, 1].sum(dtype=np.float64))
            per_img.append(ce_sum / max(cnt, 1.0))
    return np.asarray(np.mean(per_img), dtype=np.float32)
